# revision 36
# baseline (speedup 1.0000x reference)
"""DeepSets segment-reduce kernel for 8x TRN2 NeuronCores (Bass/Tile).

Computes: out = rho_mlp(segment_mean(phi_mlp(ins), batch))  for sorted
segment ids `batch` in [0, 50000), ins [1M, 128] f32.

Strategy (v2 — variable-width windows, narrow one-hot reduce):
  - Rows are packed into "windows" of whole segments with at most
    R_WIN=512 rows (4 x 128-row blocks) AND at most SPAN=32 segments.
    Row-bound almost always binds (mean segment count ~20 rows, so a
    512-row window spans ~25 segments) => ~97% packing efficiency vs
    ~90% for fixed 128-segment windows.  Windows are dealt contiguously
    to the 8 cores; one SPMD NEFF serves all cores.
  - Host preprocessing: xt = ((x + u) * 1/count)^T per window in fp16
    (u = W1^-T b1 absorbs the phi layer-1 bias exactly; the 1/count
    prescale turns the segment-sum into a mean via relu homogeneity),
    plus fp8 one-hot S per 128-row block mapping rows -> local segment
    (32 columns instead of 128 => reduce matmul streams 4x fewer cols).
  - Device, per window: 4x phi-L1 matmuls (xt block stationary, W1
    moving), relu PSUM->SBUF split across Vector (blocks 0-1) and
    Scalar (blocks 2-3), then 4x one-hot reduce matmuls accumulating
    T[hid, local_seg] in PSUM (means, since xt was prescaled).
  - phi-L2 is reassociated onto the 50k segment means (~20x less work)
    and phi_b2 is folded into rho_b1' = rho_b1 + phi_b2 @ rho_W1
    (exact: it lands pre-relu), so no b2/nonzero handling on device.
    Empty segments (none at these sizes, but handled) are fixed up on
    host.
  - Tail per 16-window group: 3 matmuls (W2^T, RW1^T, RW2^T stationary)
    with N=512 streams + scalar activations with per-partition biases,
    emitted one pipeline stage per window (deferred) so the strict
    in-order Vector/Scalar FIFOs never head-of-line block the relu.
  - Emission is software-pipelined: the reduce lags phi by LOOK=5
    windows, xt arrives in 8-window DMA superblocks prefetched 2 deep
    (big descriptors keep the 16 DMA engines ~fully fed — the kernel is
    DMA-bound at ~350 GB/s/core), and one dependency-free filler matmul
    per window keeps the PE clock's activity monitor from throttling.
    Measured: ~147-151 us vs 204 us for the previous kernel.

kernel(**inputs) takes the full unsharded inputs and returns the full
[50000, 128] fp32 output.
"""

import numpy as np
import ml_dtypes

import concourse.mybir as mybir
import concourse.tile as tile
from concourse import bacc
from concourse.bass_utils import run_bass_kernel_spmd

P = 128
N_CORES = 8
NB = 4                     # 128-row blocks per window
R_WIN = NB * P             # rows per window (512)
SPAN = 32                  # max segments per window (one-hot width)
HGW = 8                    # windows per tps PSUM tile (half tail group)
GW = 16                    # windows per tail group
N_SEGS = 50000
F16 = mybir.dt.float16
F32 = mybir.dt.float32
F8 = mybir.dt.float8e4
FP8NP = ml_dtypes.float8_e4m3
LOOK = 5                   # reduce lags phi by LOOK windows
SBW = 8                    # windows per xt DMA superblock


def _f16(a):
    return np.asarray(a, dtype=np.float32).astype(np.float16)


def _plan_windows(batch, n_segs):
    """Greedy variable-width windows: whole segments, <=R_WIN rows,
    <=SPAN segments."""
    seg_starts = np.searchsorted(batch, np.arange(n_segs + 1)).astype(np.int64)
    wins = []
    s = 0
    while s < n_segs:
        row_lo = seg_starts[s]
        e = int(np.searchsorted(seg_starts, row_lo + R_WIN, side="right")) - 1
        e = min(e, s + SPAN)
        if e <= s:
            raise ValueError("segment with more than R_WIN rows")
        wins.append((s, e))
        s = e
    return wins, seg_starts


def _host_prep(ins, batch, wts, n_segs):
    batch = np.asarray(batch).astype(np.int64)
    ins = np.asarray(ins, dtype=np.float32)
    counts = np.bincount(batch, minlength=n_segs).astype(np.float64)
    invc = np.where(counts > 0, 1.0 / np.maximum(counts, 1.0), 0.0)

    # Absorb the phi layer-1 bias into x: with u = W1f^-T b1 (solved
    # against the fp16-rounded W1), ((x+u) @ W1f) == x @ W1f + b1.
    W1f = _f16(wts["phi_W1"]).astype(np.float64)
    b1d = np.asarray(wts["phi_b1"], np.float64)
    u = np.linalg.solve(W1f.T, b1d)
    assert np.isfinite(u).all() and np.abs(u).max() < 64.0, "bias absorb failed"

    wins, seg_starts = _plan_windows(batch, n_segs)
    W_tot = len(wins)
    base, rem = divmod(W_tot, N_CORES)
    counts_c = [base + (1 if c < rem else 0) for c in range(N_CORES)]
    nwin = max(counts_c)
    starts_c = np.concatenate([[0], np.cumsum(counts_c)])

    rinv_row = invc[batch].astype(np.float32)
    scaled = ((ins + u.astype(np.float32)) * rinv_row[:, None]).astype(
        np.float16)                                   # [N, 128]

    per_core = []
    core_wins = []
    for c in range(N_CORES):
        cw = wins[starts_c[c]:starts_c[c + 1]]
        core_wins.append(cw)
        xt = np.zeros((P, nwin * R_WIN), np.float16)
        sf = np.zeros((P, nwin * NB * SPAN), FP8NP)
        if cw:
            w_cnts = np.array([seg_starts[e] - seg_starts[s] for s, e in cw])
            row_lo = seg_starts[cw[0][0]]
            row_hi = seg_starts[cw[-1][1]]
            rg = np.arange(row_lo, row_hi)
            w_of = np.repeat(np.arange(len(cw)), w_cnts)
            slot = rg - np.repeat(
                np.array([seg_starts[s] for s, _ in cw]), w_cnts)
            xt[:, w_of * R_WIN + slot] = scaled[rg].T
            blk = slot // P
            rr = slot % P
            ls = batch[rg] - np.repeat(np.array([s for s, _ in cw]), w_cnts)
            sf[rr, (w_of * NB + blk) * SPAN + ls] = 1.0
        per_core.append({"xt": xt, "sf": sf})
    return per_core, core_wins, nwin, counts, seg_starts


def _host_consts(wts):
    RW1f = _f16(wts["rho_W1"]).astype(np.float64)
    rb1p = (np.asarray(wts["rho_b1"], np.float64)
            + np.asarray(wts["phi_b2"], np.float64) @ RW1f)
    w1pack = _f16(wts["phi_W1"])
    cpack16 = np.concatenate(
        [_f16(wts["phi_W2"]),
         _f16(wts["rho_W1"]), _f16(wts["rho_W2"])], axis=1)
    cpack32 = np.stack(
        [rb1p.astype(np.float32),
         np.asarray(wts["rho_b2"], np.float32)], axis=1)
    return {"w1pack": w1pack, "cpack16": cpack16, "cpack32": cpack32}


def _build(nwin, consts_np):
    ngrp = -(-nwin // GW)
    nsb = -(-nwin // SBW)
    nc = bacc.Bacc("TRN2", target_bir_lowering=False, debug=False,
                   num_devices=N_CORES)
    d_xt = nc.dram_tensor("xt", [P, nwin * R_WIN], F16,
                          kind="ExternalInput").ap()
    d_s = nc.dram_tensor("sf", [P, nwin * NB * SPAN], F8,
                         kind="ExternalInput").ap()
    d_consts = {
        k: nc.dram_tensor(
            k, list(v.shape), mybir.dt.from_np(v.dtype), kind="ExternalInput"
        ).ap()
        for k, v in consts_np.items()
    }
    d_out = nc.dram_tensor("outT", [P, nwin * SPAN], F16,
                           kind="ExternalOutput").ap()

    with tile.TileContext(nc) as tc:
        with (
            tc.tile_pool(name="const", bufs=1) as constp,
            tc.tile_pool(name="outsb", bufs=1) as outp,
            tc.tile_pool(name="xt", bufs=4) as xtp,
            tc.tile_pool(name="sfp", bufs=4) as sfpp,
            tc.tile_pool(name="h1r", bufs=LOOK + 2) as h1rp,
            tc.tile_pool(name="tail16", bufs=2) as tailp,
            tc.tile_pool(name="h1ps", bufs=4, space="PSUM") as h1psp,
            tc.tile_pool(name="tps", bufs=2, space="PSUM") as tpsp,
            tc.tile_pool(name="tailps", bufs=1, space="PSUM") as tailpsp,
            tc.tile_pool(name="dummy", bufs=1, space="PSUM") as dummyp,
        ):
            cs_ = {}
            for k in ["w1pack", "cpack16", "cpack32"]:
                v = consts_np[k]
                cs_[k] = constp.tile(
                    list(v.shape), mybir.dt.from_np(v.dtype), name=f"c_{k}")
                nc.scalar.dma_start(cs_[k], d_consts[k])
            w1_c = cs_["w1pack"]
            w2_c = cs_["cpack16"][:, 0:128]
            rw1_c = cs_["cpack16"][:, 128:256]
            rw2_c = cs_["cpack16"][:, 256:384]
            rb1_c = cs_["cpack32"][:, 0:1]
            rb2_c = cs_["cpack32"][:, 1:2]
            outsb = outp.tile([P, nwin * SPAN], F16)

            # HAM warm-keeper: dependency-free filler matmuls keep the PE
            # clock at 8/8 (2.4 GHz); without them the PE's ~90% warm duty
            # cycle trips the activity monitor and the clock locks at
            # 1.2 GHz, making the PE the bottleneck. Rotate over 4 regions
            # of one PSUM bank to dodge drain-before-fill bubbles.
            dummy_ps = dummyp.tile([P, 4 * P], F32, space="PSUM",
                                   name="dummy_ps")
            dummy_i = [0]

            def emit_dummy(n=1):
                for _ in range(n):
                    r = dummy_i[0] % 4
                    dummy_i[0] += 1
                    nc.tensor.matmul(
                        dummy_ps[:, r * P:(r + 1) * P], lhsT=w1_c, rhs=w1_c,
                        start=True, stop=True)

            xts, sfs, h1rs, tps_t = {}, {}, {}, {}
            pending = {}            # loop-step -> [thunk, ...]

            def defer(step, fn):
                pending.setdefault(step, []).append(fn)

            def issue_xt(sb, split_first=False):
                if sb >= nsb:
                    return
                sbw = min(SBW, nwin - sb * SBW)
                t = xtp.tile([P, sbw * R_WIN], F16, name=f"xt{sb}", tag="xt",
                             padded_shape=[P, SBW * R_WIN])
                lo = sb * SBW * R_WIN
                if split_first and sbw > 1:
                    # first window lands first so phi(0) starts early
                    pieces = [1, sbw - 1]
                elif sb >= nsb - 2:
                    # fine-grained pieces at the end: compute chases the
                    # DMA stream 2 windows behind instead of 8
                    pieces = [2] * (sbw // 2) + ([1] if sbw % 2 else [])
                else:
                    # halves: smoother queue feeding, 4-window arrival
                    pieces = [sbw // 2, sbw - sbw // 2] if sbw > 1 else [sbw]
                off = 0
                for pc in pieces:
                    nc.sync.dma_start(
                        t[:, off * R_WIN:(off + pc) * R_WIN],
                        d_xt[:, lo + off * R_WIN:lo + (off + pc) * R_WIN])
                    off += pc
                xts[sb] = t

            def issue_s(g):
                if g >= ngrp:
                    return
                gsz = min(GW, nwin - g * GW)
                t = sfpp.tile([P, gsz * NB * SPAN], F8, name=f"s{g}", tag="s",
                              padded_shape=[P, GW * NB * SPAN])
                nc.scalar.dma_start(
                    t, d_s[:, g * GW * NB * SPAN:
                           (g * GW + gsz) * NB * SPAN])
                sfs[g] = t

            def emit_phi(w):
                if w % SBW == 0:
                    issue_xt(w // SBW + 2)
                if w % GW == 0:
                    issue_s(w // GW + 3)
                sb, wo = divmod(w, SBW)
                xt_w = xts[sb][:, wo * R_WIN:(wo + 1) * R_WIN]
                h1ps = h1psp.tile([P, R_WIN], F32, space="PSUM", tag="h1ps",
                                  name=f"h1ps{w}")
                for b in range(NB):
                    nc.tensor.matmul(
                        h1ps[:, b * P:(b + 1) * P],
                        lhsT=xt_w[:, b * P:(b + 1) * P], rhs=w1_c,
                        start=True, stop=True)
                if wo == SBW - 1:
                    xts.pop(sb)
                h1r = h1rp.tile([P, R_WIN], F16, tag="h1r", name=f"h1r{w}")
                # one full-window relu per engine, alternating 1:1 (V:S)
                if w % 2 == 0:
                    nc.vector.tensor_scalar(
                        h1r, h1ps, 0.0, None, op0=mybir.AluOpType.max)
                else:
                    nc.scalar.activation(
                        h1r, h1ps, mybir.ActivationFunctionType.Relu)
                h1rs[w] = h1r

            def emit_tail(g, step):
                # stages deferred so engine FIFOs never head-of-line block
                gsz = min(GW, nwin - g * GW)
                gs = gsz * SPAN
                pshape = [P, GW * SPAN]
                tps = tps_t.pop(g)
                t_sb = tailp.tile([P, gs], F16, tag="t_sb",
                                  name=f"tsb{g}", padded_shape=pshape)
                smps = tailpsp.tile([P, gs], F32, space="PSUM",
                                    tag="tailps", name=f"smps{g}",
                                    padded_shape=pshape)
                sm_sb = tailp.tile([P, gs], F16, tag="sm_sb",
                                   name=f"sm{g}", padded_shape=pshape)
                r1ps = tailpsp.tile([P, gs], F32, space="PSUM",
                                    tag="tailps", name=f"r1ps{g}",
                                    padded_shape=pshape)
                r1_sb = tailp.tile([P, gs], F16, tag="r1_sb",
                                   name=f"r1{g}", padded_shape=pshape)
                ops_ = tailpsp.tile([P, gs], F32, space="PSUM",
                                    tag="tailps", name=f"ops{g}",
                                    padded_shape=pshape)
                reg = outsb[:, g * GW * SPAN:g * GW * SPAN + gs]
                defer(step + 1, lambda: nc.vector.tensor_copy(t_sb, tps))
                defer(step + 2, lambda: nc.tensor.matmul(
                    smps, lhsT=w2_c, rhs=t_sb, start=True, stop=True))
                defer(step + 3, lambda: nc.vector.tensor_copy(sm_sb, smps))
                defer(step + 4, lambda: nc.tensor.matmul(
                    r1ps, lhsT=rw1_c, rhs=sm_sb, start=True, stop=True))
                defer(step + 5, lambda: nc.scalar.activation(
                    r1_sb, r1ps, mybir.ActivationFunctionType.Relu,
                    bias=rb1_c))
                defer(step + 6, lambda: nc.tensor.matmul(
                    ops_, lhsT=rw2_c, rhs=r1_sb, start=True, stop=True))

                def _finish():
                    nc.scalar.activation(
                        reg, ops_, mybir.ActivationFunctionType.Identity,
                        bias=rb2_c)
                    # NOT the gpsimd ring: that one lands on DMA_0 only
                    nc.scalar.dma_start(
                        d_out[:, g * GW * SPAN:g * GW * SPAN + gs], reg)
                defer(step + 7, _finish)

            def emit_red(w, step):
                g, j = divmod(w, GW)
                gsz = min(GW, nwin - g * GW)
                if j == 0:
                    tps_t[g] = tpsp.tile([P, gsz * SPAN], F32, space="PSUM",
                                         tag="tps", name=f"tps{g}",
                                         padded_shape=[P, GW * SPAN])
                tps = tps_t[g]
                st = sfs[g]
                h1r = h1rs.pop(w)
                for b in range(NB):
                    nc.tensor.matmul(
                        tps[:, j * SPAN:(j + 1) * SPAN],
                        lhsT=h1r[:, b * P:(b + 1) * P],
                        rhs=st[:, (j * NB + b) * SPAN:(j * NB + b + 1) * SPAN],
                        start=(b == 0), stop=(b == NB - 1))
                if j == gsz - 1:
                    sfs.pop(g)
                    emit_tail(g, step)
                emit_dummy()

            issue_xt(0, split_first=True)
            issue_xt(1)
            issue_s(0)
            issue_s(1)
            issue_s(2)
            emit_dummy(16)          # pre-warm the PE during startup DMAs
            for w in range(nwin + LOOK + 8):
                for fn in pending.pop(w, []):
                    fn()
                if w < nwin:
                    emit_phi(w)
                if LOOK <= w < nwin + LOOK:
                    emit_red(w - LOOK, w)

    nc.compile()
    return nc


def _gather(outs, core_wins, seg_starts, counts, wts, n_segs):
    full = np.empty((n_segs, P), np.float32)
    for c in range(N_CORES):
        cw = core_wins[c]
        if not cw:
            continue
        seg_ids = np.concatenate(
            [np.arange(s, e) for s, e in cw])
        col_ids = np.concatenate(
            [w * SPAN + np.arange(e - s) for w, (s, e) in enumerate(cw)])
        full[seg_ids] = outs[c][:, col_ids].T.astype(np.float32)
    empty = np.where(counts == 0)[0]
    if empty.size:
        rb1 = np.asarray(wts["rho_b1"], np.float64)
        row = (np.maximum(rb1, 0.0) @ np.asarray(wts["rho_W2"], np.float64)
               + np.asarray(wts["rho_b2"], np.float64))
        full[empty] = row.astype(np.float32)
    return full


def _run(inputs, n_segs=N_SEGS, trace=False, **hw_kwargs):
    ins = np.asarray(inputs["ins"])
    batch = np.asarray(inputs["batch"])
    per_core, core_wins, nwin, counts, seg_starts = _host_prep(
        ins, batch, inputs, n_segs)
    consts_np = _host_consts(inputs)
    nc = _build(nwin, consts_np)

    in_maps = []
    for c in range(N_CORES):
        m = dict(consts_np)
        m.update(per_core[c])
        in_maps.append(m)
    res = run_bass_kernel_spmd(
        nc, in_maps, core_ids=list(range(N_CORES)), trace=trace, **hw_kwargs
    )
    outs = [r["outT"] for r in res.results]           # [128, nwin*32] f16
    full = _gather(outs, core_wins, seg_starts, counts, inputs, n_segs)
    return full, res


def kernel(**inputs):
    out, _ = _run(inputs)
    return out
